# revision 44
# baseline (speedup 1.0000x reference)
"""MoE routing kernel for Trainium2 (8 NeuronCores, data-parallel over batch).

Reference computation (B=1024, PHASE=64, GATE=128, K=8, D=512):
    coeff = softmax(gateMLP(phase))                       # [B, K]
    per layer l in 0..2:
        y = sum_k coeff[:,k] * (y @ W_l[k]) + coeff @ b_l # [B, D]
        y = elu(y)  (layers 0,1 only)

Device mapping (per core, B_local = 128 rows):
  - Gate runs in transposed-activation layout (h.T = [g, b]); phase.T is
    prepared on the host, so no transposes are needed anywhere.
  - Activations carry a +1 shift: y' = elu(y)+1, with the shift absorbed
    into the next layer's bias (b' = b - W.sum(axis=in)) on the host. This
    makes the activation 3 ops: e=Exp(x), r=Relu(x), out=(e min 1)+r.
  - Softmax normalization is deferred: the expert path uses unnormalized
    e = exp(logits - max); 1/sum(e) is folded into the per-partition `scale`
    of the activation/copy that drains the layer's PSUM.
  - Layer 0 runs gate-independent per-expert matmuls (x.T is packed by the
    host) and post-scales with diag(e_k). The PE p-state ramp (~5.7us at
    half rate from the first matmul) is spent on REAL work: a minimal
    warmup block (~1us, sized to the const-DMA latency) keeps the clock
    ramping, then the gate's latency-bound matmuls are interleaved between
    the first experts' matmuls, and each expert's combine matmuls are
    interleaved behind later experts, chasing the W0 DMA stream.
  - Layers 1-2 pre-scale: z_k.T = y.T * e[:,k] via y_chunk.T @ diag(e_k)
    with 4 experts' diagonals concatenated per N=512 fp16 matmul — fusing
    transpose and per-sample scaling; then the combine runs K-MAJOR (all 4
    ic chunks of expert k, then k+1) with the two half-width PSUM banks
    staggered by one expert (L does k+1 while R does k), so each 512KB
    weight chunk is fully consumed as it lands and only ~1us of matmuls
    remain after the last chunk of the layer arrives.
  - Each layer's output accumulates COLUMN-SPLIT into two PSUM banks
    (cols 0:256 and 256:512).  The left bank stops one expert before the
    right, so the boundary ELU and the next layer's z-prep pipeline under
    the right half's matmuls; at layer 2 the left output half is copied
    and DMA'd out while the right half is still accumulating.
  - All expert-path operands are fp16 (PSUM accumulates fp32): full TensorE
    rate and half the weight DMA.  The ends are paced by: DMA stream
    (12.6 MB/core at ~400 GB/s shared HBM) for W0, then the serial PE
    chain (z-prep + combine, ~109ns per 256-wide matmul) for layers 1-2.
"""

import numpy as np

import concourse.mybir as mybir
import concourse.tile as tile
from concourse import bacc

AFT = mybir.ActivationFunctionType
ALU = mybir.AluOpType
F32 = mybir.dt.float32
F16 = mybir.dt.float16
AX = mybir.AxisListType

B, PHASE, GATE, K, D = 1024, 64, 128, 8, 512
NCORES = 8
BL = B // NCORES          # 128 rows per core
IC = D // 128             # 4 contraction chunks of 128
LW = K * IC * D           # weight columns per layer (16384)


def emit_moe(tc, out_ap, ins):
    """Emit the per-core MoE program. ins is a dict of DRAM APs."""
    nc = tc.nc

    with (
        tc.tile_pool(name="consts", bufs=1) as cpool,
        tc.tile_pool(name="ypool", bufs=2) as ypool,
        tc.tile_pool(name="zpool", bufs=2) as zpool,
        tc.tile_pool(name="tmp", bufs=3) as tpool,
        tc.tile_pool(name="ps_out", bufs=2, space="PSUM") as ps_out,
        tc.tile_pool(name="ps_z", bufs=4, space="PSUM") as ps_z,
        tc.tile_pool(name="ps_exp", bufs=2, space="PSUM") as ps_exp,
    ):
        # ---- gate/const loads on the Activation HWDGE queue ----
        t_c32 = cpool.tile([128, 530], F32)
        nc.scalar.dma_start(out=t_c32, in_=ins["c32"])
        t_ident = t_c32[:, 0:128]
        t_gw1 = t_c32[:, 128:256]
        t_gw0 = t_c32[0:PHASE, 256:384]
        t_phT = t_c32[0:PHASE, 384:512]
        t_gw2 = t_c32[:, 512:520]
        t_gb0 = t_c32[:, 520:521]
        t_gb1 = t_c32[:, 521:522]
        t_gb2 = t_c32[0:1, 522:530]

        t_wf = cpool.tile([128, 512 + 3 * LW], F16)
        t_w = t_wf[:, 512:512 + 3 * LW]

        t_c16 = cpool.tile([128, D], F16)
        nc.scalar.dma_start(out=t_c16, in_=ins["c16"])
        t_xT = t_c16[:, 0:D]              # x.T chunks: [p, ic*128+b] = x[b, ic*128+p]

        t_cb = cpool.tile([K, 3 * D], F16)
        nc.scalar.dma_start(out=t_cb, in_=ins["cb"])
        t_bias = t_cb[0:K, 0:3 * D]

        # ---- expert weights: all 3 layers resident, per-(l,k) DMAs so each
        # expert's matmuls start as soon as its 512KB chunk lands.  One
        # stream queue (the device HBM is saturated by the 8 cores' pulls);
        # W0[0] went on the scalar queue above.
        # 512 never-DMA'd pad columns at the head of t_wf: the PE warmup
        # matmuls read them (no writers -> ready at t=0).
        for l in range(3):
            for k in range(K):
                base = l * LW + k * 2048
                if l == 0:
                    # L0 chases each chunk's arrival: halve the chunks so the
                    # first two ic matmuls of each expert start ~0.65us earlier
                    nc.sync.dma_start(
                        out=t_w[:, base:base + 1024], in_=ins["W"][l, k, :, 0:1024]
                    )
                    nc.sync.dma_start(
                        out=t_w[:, base + 1024:base + 2048],
                        in_=ins["W"][l, k, :, 1024:2048],
                    )
                else:
                    nc.sync.dma_start(
                        out=t_w[:, base:base + 2048], in_=ins["W"][l, k]
                    )

        # ACT warmup: pull the activation tables off the critical path.
        t_ones = cpool.tile([1, GATE], F32)
        nc.vector.memset(t_ones, 1.0)
        t_warm = tpool.tile([1, 8], F32, tag="warm")
        nc.scalar.activation(t_warm, t_ones[:, :8], AFT.Exp)
        t_warm2 = tpool.tile([1, 8], F32, tag="warm")
        nc.scalar.activation(t_warm2, t_ones[:, :8], AFT.Relu)
        # PE ramp keep-alive: dummy matmuls on the weight tensor's
        # never-written head pad (no writers -> ready at t=0).  The DMA pipe
        # has ~4-6.5us issue-to-completion latency after the preamble: c32
        # lands ~10.9us, the first W0 chunk ~13.2us.  The PE p-state ramp
        # (half rate for ~5.7us from the first matmul, resetting on any idle
        # >~100ns) must be kept continuously busy through that window so the
        # expert stream starts at the full warmed rate exactly when its data
        # lands: a 16-mm block covers until c32, and warm() singles are woven
        # into every gate ELU-latency gap below.
        nc.vector.memset(t_wf[:, 0:512], 0.0)
        p_warm = ps_out.tile([BL, D], F32, tag="out")

        def warm(n):
            for _ in range(n):
                nc.tensor.matmul(
                    p_warm[:, 0:256], lhsT=t_wf[:, 0:128], rhs=t_wf[:, 0:256],
                    start=True, stop=True,
                )

        warm(16)

        # ---- gate + layer-0, with PE emission interleaved --------------
        # PE executes in order, so the gate's latency-bound matmuls (whose
        # ELU chains run on Scalar/Vector) are woven between the layer-0
        # gate-independent per-expert matmuls (x.T @ W0[k], paced by W-chunk
        # arrival), and each expert's diag(e_k) combine matmuls are woven
        # behind later experts' matmuls.
        t_pe = zpool.tile([128, K * D], F16, tag="z")
        p_es = {}

        def expert_mms(k):
            p_e = ps_exp.tile([128, 512], F32, tag="pexp", name=f"p_e{k}")
            for ic in range(IC):
                nc.tensor.matmul(
                    p_e,
                    lhsT=t_xT[:, ic * 128:(ic + 1) * 128],
                    rhs=t_w[:, k * 2048 + ic * 512:k * 2048 + (ic + 1) * 512],
                    start=(ic == 0),
                    stop=(ic == 3),
                )
            p_es[k] = p_e

        def expert_copy(k):
            # drain each expert's PSUM to SBUF with a half on each PSUM-
            # capable engine (GpSimd cannot access PSUM); all SBUF-only work
            # (diag builds, ELU combines) lives on GpSimd so these queues
            # stay clear — the combine matmuls chase these drains.
            dst = t_pe[:, k * 512:(k + 1) * 512]
            nc.vector.tensor_copy(out=dst[:, 0:256], in_=p_es[k][:, 0:256])
            nc.scalar.copy(dst[:, 256:512], p_es[k][:, 256:512])

        # gate stage 1 (needs only c32, landing ~10.9us) runs right after the
        # warmup block; warm() singles fill each ELU-latency gap so the PE
        # ramp never idle-resets before the experts start on W0[0] (~13.2us).
        p_g = ps_z.tile([128, 512], F32, tag="zps")
        nc.tensor.matmul(p_g[:GATE, :BL], lhsT=t_gw0, rhs=t_phT, start=True, stop=True)
        h1 = tpool.tile([GATE, BL], F32, tag="h")
        _elu1(nc, tpool, h1, p_g[:GATE, :BL], bias=t_gb0)

        warm(3)

        p_g2 = ps_z.tile([128, 512], F32, tag="zps")
        nc.tensor.matmul(p_g2[:GATE, :BL], lhsT=t_gw1, rhs=h1, start=True, stop=True)
        h2 = tpool.tile([GATE, BL], F32, tag="h")
        _elu1(nc, tpool, h2, p_g2[:GATE, :BL], bias=t_gb1)

        warm(3)

        # logits[b, k] (normal layout; gb2 via ones-row matmul)
        p_lg = ps_z.tile([128, 512], F32, tag="zps")
        nc.tensor.matmul(p_lg[:BL, :K], lhsT=h2, rhs=t_gw2, start=True, stop=False)
        nc.tensor.matmul(p_lg[:BL, :K], lhsT=t_ones, rhs=t_gb2, start=False, stop=True)

        # e = exp(logits - rowmax)   (unnormalized softmax numerator)
        t_nmx = tpool.tile([BL, 1], F32)
        nc.vector.reduce_max(t_nmx, p_lg[:BL, :K], axis=AX.X, negate=True)
        t_e = cpool.tile([BL, K], F32)
        nc.scalar.activation(t_e, p_lg[:BL, :K], AFT.Exp, bias=t_nmx, scale=1.0)

        # normalizer 1/sum(e) — consumed much later as a PSUM-drain scale
        t_sum = tpool.tile([BL, 1], F32)
        nc.vector.reduce_sum(t_sum, t_e, axis=AX.X)
        t_rcp = cpool.tile([BL, 1], F32)
        nc.vector.reciprocal(t_rcp, t_sum)

        warm(2)

        # e.T (fp16) for the mixed-bias matmul
        p_et = ps_z.tile([128, 512], F32, tag="zps")
        nc.tensor.transpose(p_et[:K, :BL], t_e, t_ident)
        t_eT = cpool.tile([K, BL], F16)
        nc.scalar.copy(t_eT, p_et[:K, :BL])

        # diag quads: [diag(e_{4q}) .. diag(e_{4q+3})], split DVE/ACT
        # (per-partition scalar pointers are not supported on Pool)
        t_diag = cpool.tile([128, 2 * 512], F16)
        for k in range(K):
            dst = t_diag[:, k * 128:(k + 1) * 128]
            sc = t_e[:, k:k + 1]
            if k % 2 == 0:
                nc.vector.tensor_scalar_mul(dst, t_ident, sc)
            else:
                nc.scalar.activation(dst, t_ident, AFT.Copy, scale=sc)

        expert_mms(0)
        expert_copy(0)
        expert_mms(1)
        expert_copy(1)
        expert_mms(2)
        expert_copy(2)
        expert_mms(3)
        expert_copy(3)

        def _l0_mm(po, k, h, start=False, stop=False):
            cs = slice(h * 256, h * 256 + 256)
            if k < 0:
                nc.tensor.matmul(
                    po[:, 0:256], lhsT=t_eT, rhs=t_bias[:, 0:D][:, cs],
                    start=False, stop=stop,
                )
            else:
                nc.tensor.matmul(
                    po[:, 0:256],
                    lhsT=t_diag[:, k * 128:(k + 1) * 128],
                    rhs=t_pe[:, k * 512:(k + 1) * 512][:, cs],
                    start=start,
                    stop=False,
                )

        # experts 4-7 chase the W0 stream; combines ride behind them with the
        # left half one expert ahead, so the L bank stops 2 matmuls + bias
        # before R and the boundary ELU-L starts under R's tail.
        p_oL = ps_out.tile([BL, D], F32, tag="out")
        p_oR = ps_out.tile([BL, D], F32, tag="out")
        for k in range(4, K):
            expert_mms(k)
            expert_copy(k)
            ck = k - 4
            _l0_mm(p_oL, ck, 0, start=(ck == 0))
            if ck > 1:
                _l0_mm(p_oR, ck - 2, 1, start=(ck == 2))
        for ck in range(4, K):
            _l0_mm(p_oL, ck, 0)
            _l0_mm(p_oR, ck - 2, 1)
        _l0_mm(p_oL, -1, 0, stop=True)
        _l0_mm(p_oR, K - 2, 1)
        _l0_mm(p_oR, K - 1, 1)
        _l0_mm(p_oR, -1, 1, stop=True)

        def warm_t(n):
            # transition filler: keep the PE ramp alive through the boundary
            # ELU window (an idle reset costs ~3us of half-rate matmuls on
            # re-entry).  Draws a fresh PSUM tile from the expert pool, which
            # is dead after layer 0 — p_warm's bank has been recycled into
            # the layer-output pool by now.
            pw = ps_exp.tile([128, 512], F32, tag="pexp")
            for _ in range(n):
                nc.tensor.matmul(
                    pw[:, 0:256], lhsT=t_wf[:, 0:128], rhs=t_wf[:, 0:256],
                    start=True, stop=True,
                )

        def _elu_q(t_e, t_r, ydst, po, h, q):
            # quarter-wide ELU into its OWN [BL,128] y tile: the next
            # layer's z matmul for this quarter then waits only a half-
            # length ELU chain after the bank's stop+flush, which the
            # stagger + warm fillers fully hide
            sl = slice(h * 256 + q * 128, h * 256 + (q + 1) * 128)
            ps = slice(q * 128, (q + 1) * 128)
            nc.scalar.activation(
                t_e[:, sl], po[:, ps], AFT.Exp, bias=0.0, scale=t_rcp
            )
            nc.vector.tensor_scalar(
                t_r[:, sl], po[:, ps], t_rcp, 0.0, op0=ALU.mult, op1=ALU.max
            )
            nc.vector.scalar_tensor_tensor(
                ydst[:, 0:128], in0=t_e[:, sl], scalar=1.0, in1=t_r[:, sl],
                op0=ALU.min, op1=ALU.add,
            )

        yq = [ypool.tile([BL, 128], F16, tag=f"y{i}", name=f"yq{i}") for i in range(4)]
        t_e0 = tpool.tile([BL, D], F32, tag="elu_e")
        t_r0 = tpool.tile([BL, D], F32, tag="elu_r")
        _elu_q(t_e0, t_r0, yq[0], p_oL, 0, 0)
        _elu_q(t_e0, t_r0, yq[1], p_oL, 0, 1)
        _elu_q(t_e0, t_r0, yq[2], p_oR, 1, 0)
        _elu_q(t_e0, t_r0, yq[3], p_oR, 1, 1)

        def _pair_mms(l, t_z, po, h, k, icp, stop=False):
            """The 2 matmuls of expert k, ic-pair icp (0 -> ic 0,1; 1 -> ic
            2,3) into half h's bank.  Pass icp=0 only needs z chunks made
            from the LEFT half of y, so combine pass 1 starts right after
            the z-L matmuls without waiting for the boundary ELU-R."""
            cs = slice(h * 256, h * 256 + 256)
            q, kq = divmod(k, 4)
            for ic in (2 * icp, 2 * icp + 1):
                nc.tensor.matmul(
                    po[:, 0:256],
                    lhsT=t_z[:, q * 2048 + ic * 512 + kq * 128:
                             q * 2048 + ic * 512 + (kq + 1) * 128],
                    rhs=t_w[:, l * LW + k * 2048 + ic * 512:
                            l * LW + k * 2048 + (ic + 1) * 512][:, cs],
                    start=False,
                    stop=(stop and ic == 2 * icp + 1),
                )

        def z_mm(t_z, y_quarters, q, ic, pool=None):
            """z_k.T = y.T * e[:,k], 4 experts' diagonals per 512-wide mm.
            Middle mms borrow the (post-L0 idle) expert PSUM banks so the
            ps_z recycle never gates them on drain completion — the drains
            queue behind the boundary ELU on Vector/Scalar and otherwise
            stretch the 1.7us z phase to ~4us."""
            p_z = (pool or ps_z).tile([128, 512], F32, tag="zps" if pool is None else "pexp")
            nc.tensor.matmul(
                p_z,
                lhsT=y_quarters[ic][:, 0:128],
                rhs=t_diag[:, q * 512:(q + 1) * 512],
                start=True,
                stop=True,
            )
            dst = t_z[:, q * 2048 + ic * 512:q * 2048 + (ic + 1) * 512]
            nc.vector.tensor_copy(out=dst[:, 0:256], in_=p_z[:, 0:256])
            nc.scalar.copy(dst[:, 256:512], p_z[:, 256:512])

        # layers 1, 2.  Per layer: the z-L matmuls (from y's ELU'd left
        # half, ready under the previous right half's tail) run first, then
        # combine pass 1 (ic 0,1 — z-L only) staggered L-ahead-by-one-k,
        # then pass 2 (ic 2,3), with the NEXT layer's z matmuls and the
        # boundary ELU woven so the PE never idles across the boundary.
        for l in range(1, 3):
            t_z = zpool.tile([128, K * D], F16, tag="z")
            # z from y-L (ELU-L completed under the previous layer's R tail)
            z_mm(t_z, yq, 0, 0)
            z_mm(t_z, yq, 0, 1)
            z_mm(t_z, yq, 1, 0)
            z_mm(t_z, yq, 1, 1)
            # z from y-R (ELU-R completes while the 4 mms above run)
            z_mm(t_z, yq, 0, 2, pool=ps_exp)
            z_mm(t_z, yq, 0, 3, pool=ps_exp)
            z_mm(t_z, yq, 1, 2)
            z_mm(t_z, yq, 1, 3)

            p_oL = ps_out.tile([BL, D], F32, tag="out")
            p_oR = ps_out.tile([BL, D], F32, tag="out")
            nc.tensor.matmul(
                p_oL[:, 0:256], lhsT=t_eT,
                rhs=t_bias[:, l * D:(l + 1) * D][:, 0:256],
                start=True, stop=False,
            )
            nc.tensor.matmul(
                p_oR[:, 0:256], lhsT=t_eT,
                rhs=t_bias[:, l * D:(l + 1) * D][:, 256:512],
                start=True, stop=False,
            )
            # k-major: expert k's 8 matmuls (both ic pairs, both halves) run
            # together so each W chunk is fully consumed as it lands and the
            # tail after the layer's last chunk is ~1us; L one expert ahead
            # of R so ELU-L hides under R's tail.
            _pair_mms(l, t_z, p_oL, 0, 0, 0)
            _pair_mms(l, t_z, p_oL, 0, 0, 1)
            for k in range(K - 1):
                _pair_mms(l, t_z, p_oL, 0, k + 1, 0)
                _pair_mms(l, t_z, p_oL, 0, k + 1, 1, stop=(k + 1 == K - 1))
                _pair_mms(l, t_z, p_oR, 1, k, 0)
                _pair_mms(l, t_z, p_oR, 1, k, 1)
            _pair_mms(l, t_z, p_oR, 1, K - 1, 0)
            _pair_mms(l, t_z, p_oR, 1, K - 1, 1, stop=True)

            if l < 2:
                yqn = [ypool.tile([BL, 128], F16, tag=f"y{i}", name=f"yqn{i}_{l}") for i in range(4)]
                t_e = tpool.tile([BL, D], F32, tag="elu_e")
                t_r = tpool.tile([BL, D], F32, tag="elu_r")
                _elu_q(t_e, t_r, yqn[0], p_oL, 0, 0)
                _elu_q(t_e, t_r, yqn[1], p_oL, 0, 1)
                _elu_q(t_e, t_r, yqn[2], p_oR, 1, 0)
                _elu_q(t_e, t_r, yqn[3], p_oR, 1, 1)
                yq = yqn
            else:
                # drain + DMA each output half as soon as its bank stops:
                # the left half's copy + DMA run under the right half's mms
                t_out = ypool.tile([BL, D], F32, tag="yout")
                nc.scalar.activation(
                    t_out[:, 0:256], p_oL[:, 0:256], AFT.Copy, scale=t_rcp
                )
                nc.sync.dma_start(out=out_ap[:, 0:256], in_=t_out[:, 0:256])
                nc.vector.tensor_scalar(
                    t_out[:, 256:384], p_oR[:, 0:128], t_rcp, 0.0,
                    op0=ALU.mult, op1=ALU.bypass,
                )
                nc.scalar.activation(
                    t_out[:, 384:512], p_oR[:, 128:256], AFT.Copy, scale=t_rcp
                )
                nc.sync.dma_start(out=out_ap[:, 256:512], in_=t_out[:, 256:512])


def _elu1(nc, tpool, out, pre, bias):
    """out = elu(pre + bias) + 1 = relu(x) + min(exp(x), 1); x = pre + bias.
    exp on Scalar, relu on Vector so the two run in parallel."""
    shape = [pre.partition_size(), pre.free_size()]
    t_e = tpool.tile(shape, F32, tag="elu_e")
    nc.scalar.activation(t_e, pre, AFT.Exp, bias=bias, scale=1.0)
    t_r = tpool.tile(shape, F32, tag="elu_r")
    nc.vector.tensor_scalar(t_r, pre, bias, 0.0, op0=ALU.add, op1=ALU.max)
    nc.vector.scalar_tensor_tensor(
        out, in0=t_e, scalar=1.0, in1=t_r, op0=ALU.min, op1=ALU.add
    )


def _prep_host(x, phase, gw0, gb0, gw1, gb1, gw2, gb2, W0, b0, W1, b1, W2, b2):
    """Host-side packing. Returns per-core input maps."""
    f32 = np.float32

    # weights blob: [3, 8, 128, 2048]; [l, k, p, ic*512 + o] = W_l[k, ic*128+p, o]
    W = np.stack([W0, W1, W2]).astype(f32)  # [3, 8, 512, 512]
    Wb = (
        W.reshape(3, K, IC, 128, D)
        .transpose(0, 1, 3, 2, 4)
        .reshape(3, K, 128, IC * D)
        .astype(np.float16)
    )
    # +1-shift corrections: layer l>0 consumes y'+1, gate layers 1,2 consume h'+1
    b0a = np.asarray(b0, f32)
    b1a = np.asarray(b1, f32) - np.asarray(W1, f32).sum(axis=1)
    b2a = np.asarray(b2, f32) - np.asarray(W2, f32).sum(axis=1)
    eb = np.concatenate([b0a, b1a, b2a], axis=1).astype(np.float16)  # [8, 1536]
    gb1a = np.asarray(gb1, f32) - np.asarray(gw1, f32).sum(axis=0)
    gb2a = np.asarray(gb2, f32) - np.asarray(gw2, f32).sum(axis=0)

    # packed fp32 const blob [128, 530]:
    #   0:128 ident | 128:256 gw1 | 256:384 gw0 (rows 0:64)
    #   | 384:512 ph.T (rows 0:64) | 512:520 gw2 | 520 gb0 | 521 gb1
    #   | 522:530 gb2 (row 0)
    c32 = np.zeros((128, 530), f32)
    c32[:, 0:128] = np.eye(128, dtype=f32)
    c32[:, 128:256] = np.asarray(gw1, f32)
    c32[0:PHASE, 256:384] = np.asarray(gw0, f32)
    c32[:, 512:520] = np.asarray(gw2, f32)
    c32[:, 520] = np.asarray(gb0, f32)
    c32[:, 521] = gb1a
    c32[0, 522:530] = gb2a

    per_core = []
    for c in range(NCORES):
        sl = slice(c * BL, (c + 1) * BL)
        cc32 = c32.copy()
        cc32[0:PHASE, 384:512] = np.asarray(phase[sl], f32).T
        # c16 [128, 512]: x.T chunks ([p, ic*128+b] = x[b, ic*128+p])
        xs = np.asarray(x[sl]).astype(np.float16)
        c16 = xs.T.reshape(IC, 128, BL).transpose(1, 0, 2).reshape(128, IC * BL)
        per_core.append(
            {
                "c32": np.ascontiguousarray(cc32),
                "c16": np.ascontiguousarray(c16),
                "cb": np.ascontiguousarray(eb),
                "W": Wb,
            }
        )
    return per_core


def _declare_dram(nc):
    f32 = mybir.dt.float32
    ins = {
        "c32": nc.dram_tensor("c32", [128, 530], f32, kind="ExternalInput").ap(),
        "c16": nc.dram_tensor("c16", [128, D], F16, kind="ExternalInput").ap(),
        "cb": nc.dram_tensor("cb", [K, 3 * D], F16, kind="ExternalInput").ap(),
        "W": nc.dram_tensor("W", [3, K, 128, IC * D], F16, kind="ExternalInput").ap(),
    }
    out = nc.dram_tensor("out", [BL, D], f32, kind="ExternalOutput").ap()
    return ins, out


_CACHED = None


def _build():
    global _CACHED
    if _CACHED is None:
        nc = bacc.Bacc(
            "TRN2", target_bir_lowering=False, debug=False, num_devices=NCORES
        )
        ins, out = _declare_dram(nc)
        with tile.TileContext(nc) as tc:
            emit_moe(tc, out, ins)
        nc.compile()
        _CACHED = nc
    return _CACHED


def kernel(**inputs) -> np.ndarray:
    from concourse.bass_utils import run_bass_kernel_spmd

    per_core = _prep_host(**inputs)
    nc = _build()
    res = run_bass_kernel_spmd(nc, per_core, core_ids=list(range(NCORES)))
    return np.concatenate([r["out"] for r in res.results], axis=0)


if __name__ == "__main__":
    import reference

    inp = {k: np.asarray(v) for k, v in reference.setup_inputs().items()}
    got = kernel(**inp)
    exp = np.asarray(reference.reference(**inp))
    err = np.abs(got - exp).max() / np.abs(exp).max()
    print("Relative error:", err)


# revision 45
# speedup vs baseline: 1.0044x; 1.0044x over previous
"""MoE routing kernel for Trainium2 (8 NeuronCores, data-parallel over batch).

Reference computation (B=1024, PHASE=64, GATE=128, K=8, D=512):
    coeff = softmax(gateMLP(phase))                       # [B, K]
    per layer l in 0..2:
        y = sum_k coeff[:,k] * (y @ W_l[k]) + coeff @ b_l # [B, D]
        y = elu(y)  (layers 0,1 only)

Device mapping (per core, B_local = 128 rows):
  - Gate runs in transposed-activation layout (h.T = [g, b]); phase.T is
    prepared on the host, so no transposes are needed anywhere.
  - Activations carry a +1 shift: y' = elu(y)+1, with the shift absorbed
    into the next layer's bias (b' = b - W.sum(axis=in)) on the host. This
    makes the activation 3 ops: e=Exp(x), r=Relu(x), out=(e min 1)+r.
  - Softmax normalization is deferred: the expert path uses unnormalized
    e = exp(logits - max); 1/sum(e) is folded into the per-partition `scale`
    of the activation/copy that drains the layer's PSUM.
  - Layer 0 runs gate-independent per-expert matmuls (x.T is packed by the
    host) and post-scales with diag(e_k). The PE p-state ramp (~5.7us at
    half rate from the first matmul) is spent on REAL work: a minimal
    warmup block (~1us, sized to the const-DMA latency) keeps the clock
    ramping, then the gate's latency-bound matmuls are interleaved between
    the first experts' matmuls, and each expert's combine matmuls are
    interleaved behind later experts, chasing the W0 DMA stream.
  - Layers 1-2 pre-scale: z_k.T = y.T * e[:,k] via y_chunk.T @ diag(e_k)
    with 4 experts' diagonals concatenated per N=512 fp16 matmul — fusing
    transpose and per-sample scaling; then the combine runs K-MAJOR (all 4
    ic chunks of expert k, then k+1) with the two half-width PSUM banks
    staggered by one expert (L does k+1 while R does k), so each 512KB
    weight chunk is fully consumed as it lands and only ~1us of matmuls
    remain after the last chunk of the layer arrives.
  - Each layer's output accumulates COLUMN-SPLIT into two PSUM banks
    (cols 0:256 and 256:512).  The left bank stops one expert before the
    right, so the boundary ELU and the next layer's z-prep pipeline under
    the right half's matmuls; at layer 2 the left output half is copied
    and DMA'd out while the right half is still accumulating.
  - All expert-path operands are fp16 (PSUM accumulates fp32): full TensorE
    rate and half the weight DMA.  The ends are paced by: DMA stream
    (12.6 MB/core at ~400 GB/s shared HBM) for W0, then the serial PE
    chain (z-prep + combine, ~109ns per 256-wide matmul) for layers 1-2.
"""

import numpy as np

import concourse.mybir as mybir
import concourse.tile as tile
from concourse import bacc

AFT = mybir.ActivationFunctionType
ALU = mybir.AluOpType
F32 = mybir.dt.float32
F16 = mybir.dt.float16
AX = mybir.AxisListType

B, PHASE, GATE, K, D = 1024, 64, 128, 8, 512
NCORES = 8
BL = B // NCORES          # 128 rows per core
IC = D // 128             # 4 contraction chunks of 128
LW = K * IC * D           # weight columns per layer (16384)


def emit_moe(tc, out_ap, ins):
    """Emit the per-core MoE program. ins is a dict of DRAM APs."""
    nc = tc.nc

    with (
        tc.tile_pool(name="consts", bufs=1) as cpool,
        tc.tile_pool(name="ypool", bufs=2) as ypool,
        tc.tile_pool(name="zpool", bufs=2) as zpool,
        tc.tile_pool(name="tmp", bufs=3) as tpool,
        tc.tile_pool(name="ps_out", bufs=2, space="PSUM") as ps_out,
        tc.tile_pool(name="ps_z", bufs=4, space="PSUM") as ps_z,
        tc.tile_pool(name="ps_exp", bufs=2, space="PSUM") as ps_exp,
    ):
        # ---- gate/const loads on the Activation HWDGE queue.  W0[0]
        # rides this queue ahead of the gate consts: its first completion
        # is ~2us faster than the sync stream's, so expert 0 runs before
        # the sync queue's first chunk lands and every later expert k
        # consumes sync chunk k-1 — a ~1us head start for layer 0. ----
        t_wf = cpool.tile([128, 512 + 3 * LW], F16)
        t_w = t_wf[:, 512:512 + 3 * LW]
        nc.scalar.dma_start(out=t_w[:, 0:1024], in_=ins["W"][0, 0, :, 0:1024])

        t_c16 = cpool.tile([128, D], F16)
        nc.scalar.dma_start(out=t_c16, in_=ins["c16"])
        t_xT = t_c16[:, 0:D]              # x.T chunks: [p, ic*128+b] = x[b, ic*128+p]

        t_c32 = cpool.tile([128, 530], F32)
        nc.scalar.dma_start(out=t_c32, in_=ins["c32"])
        t_ident = t_c32[:, 0:128]
        t_gw1 = t_c32[:, 128:256]
        t_gw0 = t_c32[0:PHASE, 256:384]
        t_phT = t_c32[0:PHASE, 384:512]
        t_gw2 = t_c32[:, 512:520]
        t_gb0 = t_c32[:, 520:521]
        t_gb1 = t_c32[:, 521:522]
        t_gb2 = t_c32[0:1, 522:530]

        nc.scalar.dma_start(
            out=t_w[:, 1024:2048], in_=ins["W"][0, 0, :, 1024:2048]
        )

        t_cb = cpool.tile([K, 3 * D], F16)
        nc.scalar.dma_start(out=t_cb, in_=ins["cb"])
        t_bias = t_cb[0:K, 0:3 * D]

        # ---- expert weights: all 3 layers resident, per-(l,k) DMAs so each
        # expert's matmuls start as soon as its 512KB chunk lands.  One
        # stream queue (the device HBM is saturated by the 8 cores' pulls);
        # W0[0] went on the scalar queue above.
        # 512 never-DMA'd pad columns at the head of t_wf: the PE warmup
        # matmuls read them (no writers -> ready at t=0).
        for l in range(3):
            for k in range(K):
                if l == 0 and k == 0:
                    continue
                base = l * LW + k * 2048
                if l == 0:
                    # L0 chases each chunk's arrival: halve the chunks so the
                    # first two ic matmuls of each expert start ~0.65us earlier
                    nc.sync.dma_start(
                        out=t_w[:, base:base + 1024], in_=ins["W"][l, k, :, 0:1024]
                    )
                    nc.sync.dma_start(
                        out=t_w[:, base + 1024:base + 2048],
                        in_=ins["W"][l, k, :, 1024:2048],
                    )
                else:
                    nc.sync.dma_start(
                        out=t_w[:, base:base + 2048], in_=ins["W"][l, k]
                    )

        # ACT warmup: pull the activation tables off the critical path.
        t_ones = cpool.tile([1, GATE], F32)
        nc.vector.memset(t_ones, 1.0)
        t_warm = tpool.tile([1, 8], F32, tag="warm")
        nc.scalar.activation(t_warm, t_ones[:, :8], AFT.Exp)
        t_warm2 = tpool.tile([1, 8], F32, tag="warm")
        nc.scalar.activation(t_warm2, t_ones[:, :8], AFT.Relu)
        # PE ramp keep-alive: dummy matmuls on the weight tensor's
        # never-written head pad (no writers -> ready at t=0).  The DMA pipe
        # has ~4-6.5us issue-to-completion latency after the preamble: c32
        # lands ~10.9us, the first W0 chunk ~13.2us.  The PE p-state ramp
        # (half rate for ~5.7us from the first matmul, resetting on any idle
        # >~100ns) must be kept continuously busy through that window so the
        # expert stream starts at the full warmed rate exactly when its data
        # lands: a 16-mm block covers until c32, and warm() singles are woven
        # into every gate ELU-latency gap below.
        nc.vector.memset(t_wf[:, 0:512], 0.0)
        p_warm = ps_out.tile([BL, D], F32, tag="out")

        def warm(n):
            for _ in range(n):
                nc.tensor.matmul(
                    p_warm[:, 0:256], lhsT=t_wf[:, 0:128], rhs=t_wf[:, 0:256],
                    start=True, stop=True,
                )

        warm(16)

        # ---- gate + layer-0, with PE emission interleaved --------------
        # PE executes in order, so the gate's latency-bound matmuls (whose
        # ELU chains run on Scalar/Vector) are woven between the layer-0
        # gate-independent per-expert matmuls (x.T @ W0[k], paced by W-chunk
        # arrival), and each expert's diag(e_k) combine matmuls are woven
        # behind later experts' matmuls.
        t_pe = zpool.tile([128, K * D], F16, tag="z")
        p_es = {}

        def expert_mms(k):
            p_e = ps_exp.tile([128, 512], F32, tag="pexp", name=f"p_e{k}")
            for ic in range(IC):
                nc.tensor.matmul(
                    p_e,
                    lhsT=t_xT[:, ic * 128:(ic + 1) * 128],
                    rhs=t_w[:, k * 2048 + ic * 512:k * 2048 + (ic + 1) * 512],
                    start=(ic == 0),
                    stop=(ic == 3),
                )
            p_es[k] = p_e

        def expert_copy(k):
            # drain each expert's PSUM to SBUF with a half on each PSUM-
            # capable engine (GpSimd cannot access PSUM); all SBUF-only work
            # (diag builds, ELU combines) lives on GpSimd so these queues
            # stay clear — the combine matmuls chase these drains.
            dst = t_pe[:, k * 512:(k + 1) * 512]
            nc.vector.tensor_copy(out=dst[:, 0:256], in_=p_es[k][:, 0:256])
            nc.scalar.copy(dst[:, 256:512], p_es[k][:, 256:512])

        # gate stage 1 (needs only c32, landing ~10.9us) runs right after the
        # warmup block; warm() singles fill each ELU-latency gap so the PE
        # ramp never idle-resets before the experts start on W0[0] (~13.2us).
        p_g = ps_z.tile([128, 512], F32, tag="zps")
        nc.tensor.matmul(p_g[:GATE, :BL], lhsT=t_gw0, rhs=t_phT, start=True, stop=True)
        h1 = tpool.tile([GATE, BL], F32, tag="h")
        _elu1(nc, tpool, h1, p_g[:GATE, :BL], bias=t_gb0)

        warm(3)

        p_g2 = ps_z.tile([128, 512], F32, tag="zps")
        nc.tensor.matmul(p_g2[:GATE, :BL], lhsT=t_gw1, rhs=h1, start=True, stop=True)
        h2 = tpool.tile([GATE, BL], F32, tag="h")
        _elu1(nc, tpool, h2, p_g2[:GATE, :BL], bias=t_gb1)

        warm(3)

        # logits[b, k] (normal layout; gb2 via ones-row matmul)
        p_lg = ps_z.tile([128, 512], F32, tag="zps")
        nc.tensor.matmul(p_lg[:BL, :K], lhsT=h2, rhs=t_gw2, start=True, stop=False)
        nc.tensor.matmul(p_lg[:BL, :K], lhsT=t_ones, rhs=t_gb2, start=False, stop=True)

        # e = exp(logits - rowmax)   (unnormalized softmax numerator)
        t_nmx = tpool.tile([BL, 1], F32)
        nc.vector.reduce_max(t_nmx, p_lg[:BL, :K], axis=AX.X, negate=True)
        t_e = cpool.tile([BL, K], F32)
        nc.scalar.activation(t_e, p_lg[:BL, :K], AFT.Exp, bias=t_nmx, scale=1.0)

        # normalizer 1/sum(e) — consumed much later as a PSUM-drain scale
        t_sum = tpool.tile([BL, 1], F32)
        nc.vector.reduce_sum(t_sum, t_e, axis=AX.X)
        t_rcp = cpool.tile([BL, 1], F32)
        nc.vector.reciprocal(t_rcp, t_sum)

        warm(2)

        # e.T (fp16) for the mixed-bias matmul
        p_et = ps_z.tile([128, 512], F32, tag="zps")
        nc.tensor.transpose(p_et[:K, :BL], t_e, t_ident)
        t_eT = cpool.tile([K, BL], F16)
        nc.scalar.copy(t_eT, p_et[:K, :BL])

        # diag quads: [diag(e_{4q}) .. diag(e_{4q+3})], split DVE/ACT
        # (per-partition scalar pointers are not supported on Pool)
        t_diag = cpool.tile([128, 2 * 512], F16)
        for k in range(K):
            dst = t_diag[:, k * 128:(k + 1) * 128]
            sc = t_e[:, k:k + 1]
            if k % 2 == 0:
                nc.vector.tensor_scalar_mul(dst, t_ident, sc)
            else:
                nc.scalar.activation(dst, t_ident, AFT.Copy, scale=sc)

        expert_mms(0)
        expert_copy(0)
        expert_mms(1)
        expert_copy(1)
        expert_mms(2)
        expert_copy(2)
        expert_mms(3)
        expert_copy(3)

        def _l0_mm(po, k, h, start=False, stop=False):
            cs = slice(h * 256, h * 256 + 256)
            if k < 0:
                nc.tensor.matmul(
                    po[:, 0:256], lhsT=t_eT, rhs=t_bias[:, 0:D][:, cs],
                    start=False, stop=stop,
                )
            else:
                nc.tensor.matmul(
                    po[:, 0:256],
                    lhsT=t_diag[:, k * 128:(k + 1) * 128],
                    rhs=t_pe[:, k * 512:(k + 1) * 512][:, cs],
                    start=start,
                    stop=False,
                )

        # experts 4-7 chase the W0 stream; combines ride behind them with the
        # left half one expert ahead, so the L bank stops 2 matmuls + bias
        # before R and the boundary ELU-L starts under R's tail.
        p_oL = ps_out.tile([BL, D], F32, tag="out")
        p_oR = ps_out.tile([BL, D], F32, tag="out")
        for k in range(4, K):
            expert_mms(k)
            expert_copy(k)
            ck = k - 4
            _l0_mm(p_oL, ck, 0, start=(ck == 0))
            if ck > 1:
                _l0_mm(p_oR, ck - 2, 1, start=(ck == 2))
        for ck in range(4, K):
            _l0_mm(p_oL, ck, 0)
            _l0_mm(p_oR, ck - 2, 1)
        _l0_mm(p_oL, -1, 0, stop=True)
        _l0_mm(p_oR, K - 2, 1)
        _l0_mm(p_oR, K - 1, 1)
        _l0_mm(p_oR, -1, 1, stop=True)

        def warm_t(n):
            # transition filler: keep the PE ramp alive through the boundary
            # ELU window (an idle reset costs ~3us of half-rate matmuls on
            # re-entry).  Draws a fresh PSUM tile from the expert pool, which
            # is dead after layer 0 — p_warm's bank has been recycled into
            # the layer-output pool by now.
            pw = ps_exp.tile([128, 512], F32, tag="pexp")
            for _ in range(n):
                nc.tensor.matmul(
                    pw[:, 0:256], lhsT=t_wf[:, 0:128], rhs=t_wf[:, 0:256],
                    start=True, stop=True,
                )

        def _elu_q(t_e, t_r, ydst, po, h, q):
            # quarter-wide ELU into its OWN [BL,128] y tile: the next
            # layer's z matmul for this quarter then waits only a half-
            # length ELU chain after the bank's stop+flush, which the
            # stagger + warm fillers fully hide
            sl = slice(h * 256 + q * 128, h * 256 + (q + 1) * 128)
            ps = slice(q * 128, (q + 1) * 128)
            nc.scalar.activation(
                t_e[:, sl], po[:, ps], AFT.Exp, bias=0.0, scale=t_rcp
            )
            nc.vector.tensor_scalar(
                t_r[:, sl], po[:, ps], t_rcp, 0.0, op0=ALU.mult, op1=ALU.max
            )
            nc.vector.scalar_tensor_tensor(
                ydst[:, 0:128], in0=t_e[:, sl], scalar=1.0, in1=t_r[:, sl],
                op0=ALU.min, op1=ALU.add,
            )

        yq = [ypool.tile([BL, 128], F16, tag=f"y{i}", name=f"yq{i}") for i in range(4)]
        t_e0 = tpool.tile([BL, D], F32, tag="elu_e")
        t_r0 = tpool.tile([BL, D], F32, tag="elu_r")
        _elu_q(t_e0, t_r0, yq[0], p_oL, 0, 0)
        _elu_q(t_e0, t_r0, yq[1], p_oL, 0, 1)
        _elu_q(t_e0, t_r0, yq[2], p_oR, 1, 0)
        _elu_q(t_e0, t_r0, yq[3], p_oR, 1, 1)

        def _pair_mms(l, t_z, po, h, k, icp, stop=False):
            """The 2 matmuls of expert k, ic-pair icp (0 -> ic 0,1; 1 -> ic
            2,3) into half h's bank.  Pass icp=0 only needs z chunks made
            from the LEFT half of y, so combine pass 1 starts right after
            the z-L matmuls without waiting for the boundary ELU-R."""
            cs = slice(h * 256, h * 256 + 256)
            q, kq = divmod(k, 4)
            for ic in (2 * icp, 2 * icp + 1):
                nc.tensor.matmul(
                    po[:, 0:256],
                    lhsT=t_z[:, q * 2048 + ic * 512 + kq * 128:
                             q * 2048 + ic * 512 + (kq + 1) * 128],
                    rhs=t_w[:, l * LW + k * 2048 + ic * 512:
                            l * LW + k * 2048 + (ic + 1) * 512][:, cs],
                    start=False,
                    stop=(stop and ic == 2 * icp + 1),
                )

        def z_mm(t_z, y_quarters, q, ic, pool=None):
            """z_k.T = y.T * e[:,k], 4 experts' diagonals per 512-wide mm.
            Middle mms borrow the (post-L0 idle) expert PSUM banks so the
            ps_z recycle never gates them on drain completion — the drains
            queue behind the boundary ELU on Vector/Scalar and otherwise
            stretch the 1.7us z phase to ~4us."""
            p_z = (pool or ps_z).tile([128, 512], F32, tag="zps" if pool is None else "pexp")
            nc.tensor.matmul(
                p_z,
                lhsT=y_quarters[ic][:, 0:128],
                rhs=t_diag[:, q * 512:(q + 1) * 512],
                start=True,
                stop=True,
            )
            dst = t_z[:, q * 2048 + ic * 512:q * 2048 + (ic + 1) * 512]
            nc.vector.tensor_copy(out=dst[:, 0:256], in_=p_z[:, 0:256])
            nc.scalar.copy(dst[:, 256:512], p_z[:, 256:512])

        # layers 1, 2.  Per layer: the z-L matmuls (from y's ELU'd left
        # half, ready under the previous right half's tail) run first, then
        # combine pass 1 (ic 0,1 — z-L only) staggered L-ahead-by-one-k,
        # then pass 2 (ic 2,3), with the NEXT layer's z matmuls and the
        # boundary ELU woven so the PE never idles across the boundary.
        for l in range(1, 3):
            t_z = zpool.tile([128, K * D], F16, tag="z")
            # z from y-L (ELU-L completed under the previous layer's R tail)
            z_mm(t_z, yq, 0, 0)
            z_mm(t_z, yq, 0, 1)
            z_mm(t_z, yq, 1, 0)
            z_mm(t_z, yq, 1, 1)
            # z from y-R (ELU-R completes while the 4 mms above run)
            z_mm(t_z, yq, 0, 2, pool=ps_exp)
            z_mm(t_z, yq, 0, 3, pool=ps_exp)
            z_mm(t_z, yq, 1, 2)
            z_mm(t_z, yq, 1, 3)

            p_oL = ps_out.tile([BL, D], F32, tag="out")
            p_oR = ps_out.tile([BL, D], F32, tag="out")
            nc.tensor.matmul(
                p_oL[:, 0:256], lhsT=t_eT,
                rhs=t_bias[:, l * D:(l + 1) * D][:, 0:256],
                start=True, stop=False,
            )
            nc.tensor.matmul(
                p_oR[:, 0:256], lhsT=t_eT,
                rhs=t_bias[:, l * D:(l + 1) * D][:, 256:512],
                start=True, stop=False,
            )
            # k-major: expert k's 8 matmuls (both ic pairs, both halves) run
            # together so each W chunk is fully consumed as it lands and the
            # tail after the layer's last chunk is ~1us; L one expert ahead
            # of R so ELU-L hides under R's tail.
            _pair_mms(l, t_z, p_oL, 0, 0, 0)
            _pair_mms(l, t_z, p_oL, 0, 0, 1)
            for k in range(K - 1):
                _pair_mms(l, t_z, p_oL, 0, k + 1, 0)
                _pair_mms(l, t_z, p_oL, 0, k + 1, 1, stop=(k + 1 == K - 1))
                _pair_mms(l, t_z, p_oR, 1, k, 0)
                _pair_mms(l, t_z, p_oR, 1, k, 1)
            _pair_mms(l, t_z, p_oR, 1, K - 1, 0)
            _pair_mms(l, t_z, p_oR, 1, K - 1, 1, stop=True)

            if l < 2:
                yqn = [ypool.tile([BL, 128], F16, tag=f"y{i}", name=f"yqn{i}_{l}") for i in range(4)]
                t_e = tpool.tile([BL, D], F32, tag="elu_e")
                t_r = tpool.tile([BL, D], F32, tag="elu_r")
                _elu_q(t_e, t_r, yqn[0], p_oL, 0, 0)
                _elu_q(t_e, t_r, yqn[1], p_oL, 0, 1)
                _elu_q(t_e, t_r, yqn[2], p_oR, 1, 0)
                _elu_q(t_e, t_r, yqn[3], p_oR, 1, 1)
                yq = yqn
            else:
                # drain + DMA each output half as soon as its bank stops:
                # the left half's copy + DMA run under the right half's mms
                t_out = ypool.tile([BL, D], F32, tag="yout")
                nc.scalar.activation(
                    t_out[:, 0:256], p_oL[:, 0:256], AFT.Copy, scale=t_rcp
                )
                nc.sync.dma_start(out=out_ap[:, 0:256], in_=t_out[:, 0:256])
                nc.vector.tensor_scalar(
                    t_out[:, 256:384], p_oR[:, 0:128], t_rcp, 0.0,
                    op0=ALU.mult, op1=ALU.bypass,
                )
                nc.scalar.activation(
                    t_out[:, 384:512], p_oR[:, 128:256], AFT.Copy, scale=t_rcp
                )
                nc.sync.dma_start(out=out_ap[:, 256:512], in_=t_out[:, 256:512])


def _elu1(nc, tpool, out, pre, bias):
    """out = elu(pre + bias) + 1 = relu(x) + min(exp(x), 1); x = pre + bias.
    exp on Scalar, relu on Vector so the two run in parallel."""
    shape = [pre.partition_size(), pre.free_size()]
    t_e = tpool.tile(shape, F32, tag="elu_e")
    nc.scalar.activation(t_e, pre, AFT.Exp, bias=bias, scale=1.0)
    t_r = tpool.tile(shape, F32, tag="elu_r")
    nc.vector.tensor_scalar(t_r, pre, bias, 0.0, op0=ALU.add, op1=ALU.max)
    nc.vector.scalar_tensor_tensor(
        out, in0=t_e, scalar=1.0, in1=t_r, op0=ALU.min, op1=ALU.add
    )


def _prep_host(x, phase, gw0, gb0, gw1, gb1, gw2, gb2, W0, b0, W1, b1, W2, b2):
    """Host-side packing. Returns per-core input maps."""
    f32 = np.float32

    # weights blob: [3, 8, 128, 2048]; [l, k, p, ic*512 + o] = W_l[k, ic*128+p, o]
    W = np.stack([W0, W1, W2]).astype(f32)  # [3, 8, 512, 512]
    Wb = (
        W.reshape(3, K, IC, 128, D)
        .transpose(0, 1, 3, 2, 4)
        .reshape(3, K, 128, IC * D)
        .astype(np.float16)
    )
    # +1-shift corrections: layer l>0 consumes y'+1, gate layers 1,2 consume h'+1
    b0a = np.asarray(b0, f32)
    b1a = np.asarray(b1, f32) - np.asarray(W1, f32).sum(axis=1)
    b2a = np.asarray(b2, f32) - np.asarray(W2, f32).sum(axis=1)
    eb = np.concatenate([b0a, b1a, b2a], axis=1).astype(np.float16)  # [8, 1536]
    gb1a = np.asarray(gb1, f32) - np.asarray(gw1, f32).sum(axis=0)
    gb2a = np.asarray(gb2, f32) - np.asarray(gw2, f32).sum(axis=0)

    # packed fp32 const blob [128, 530]:
    #   0:128 ident | 128:256 gw1 | 256:384 gw0 (rows 0:64)
    #   | 384:512 ph.T (rows 0:64) | 512:520 gw2 | 520 gb0 | 521 gb1
    #   | 522:530 gb2 (row 0)
    c32 = np.zeros((128, 530), f32)
    c32[:, 0:128] = np.eye(128, dtype=f32)
    c32[:, 128:256] = np.asarray(gw1, f32)
    c32[0:PHASE, 256:384] = np.asarray(gw0, f32)
    c32[:, 512:520] = np.asarray(gw2, f32)
    c32[:, 520] = np.asarray(gb0, f32)
    c32[:, 521] = gb1a
    c32[0, 522:530] = gb2a

    per_core = []
    for c in range(NCORES):
        sl = slice(c * BL, (c + 1) * BL)
        cc32 = c32.copy()
        cc32[0:PHASE, 384:512] = np.asarray(phase[sl], f32).T
        # c16 [128, 512]: x.T chunks ([p, ic*128+b] = x[b, ic*128+p])
        xs = np.asarray(x[sl]).astype(np.float16)
        c16 = xs.T.reshape(IC, 128, BL).transpose(1, 0, 2).reshape(128, IC * BL)
        per_core.append(
            {
                "c32": np.ascontiguousarray(cc32),
                "c16": np.ascontiguousarray(c16),
                "cb": np.ascontiguousarray(eb),
                "W": Wb,
            }
        )
    return per_core


def _declare_dram(nc):
    f32 = mybir.dt.float32
    ins = {
        "c32": nc.dram_tensor("c32", [128, 530], f32, kind="ExternalInput").ap(),
        "c16": nc.dram_tensor("c16", [128, D], F16, kind="ExternalInput").ap(),
        "cb": nc.dram_tensor("cb", [K, 3 * D], F16, kind="ExternalInput").ap(),
        "W": nc.dram_tensor("W", [3, K, 128, IC * D], F16, kind="ExternalInput").ap(),
    }
    out = nc.dram_tensor("out", [BL, D], f32, kind="ExternalOutput").ap()
    return ins, out


_CACHED = None


def _build():
    global _CACHED
    if _CACHED is None:
        nc = bacc.Bacc(
            "TRN2", target_bir_lowering=False, debug=False, num_devices=NCORES
        )
        ins, out = _declare_dram(nc)
        with tile.TileContext(nc) as tc:
            emit_moe(tc, out, ins)
        nc.compile()
        _CACHED = nc
    return _CACHED


def kernel(**inputs) -> np.ndarray:
    from concourse.bass_utils import run_bass_kernel_spmd

    per_core = _prep_host(**inputs)
    nc = _build()
    res = run_bass_kernel_spmd(nc, per_core, core_ids=list(range(NCORES)))
    return np.concatenate([r["out"] for r in res.results], axis=0)


if __name__ == "__main__":
    import reference

    inp = {k: np.asarray(v) for k, v in reference.setup_inputs().items()}
    got = kernel(**inp)
    exp = np.asarray(reference.reference(**inp))
    err = np.abs(got - exp).max() / np.abs(exp).max()
    print("Relative error:", err)
